# revision 13
# baseline (speedup 1.0000x reference)
"""Trainium2 Bass kernel for nn_PolarTransform (histogram binning + bilinear resize).

Computation (matches the reference):
    xs  = x.reshape(48, 65536)
    ws  = xs @ pooling_masks.T                 # [48, 2048]  <- the heavy part
    out = ws.reshape(16,3,32,64) / counts
    out = reflect-pad angles (1,1) -> [16,3,32,66]
    out = bilinear resize -> [16,3,128,128]

Strategy:
  * The dominant cost is streaming pooling_masks ([2048, 65536] f32 = 512 MB).
    Mask values are exactly {0.0, 1.0}, so a bf16 cast is lossless and halves
    the HBM traffic.  x is split exactly into hi+lo bf16 (x == hi + lo in f32),
    keeping ~f32 precision while using the fast bf16 matmul path (1 cyc/row
    vs 4 cyc/row for f32 on the PE).
  * Shard the 65536-pixel contraction dim across the 8 cores (8192 px each).
    Each core streams its [8192, 2048] bf16 transposed-mask slab (32 MB) and
    accumulates a partial ws [48, 2048] in PSUM.  Host sums the 8 partials
    and applies the tiny counts/pad/resize epilogue ([48, 2048] -> output).
  * Mask slabs are pre-transposed on the host (constant-buffer pre-packing) so
    the device streams them at DMA line rate with the contraction dim on
    SBUF partitions, as the PE requires.
"""

import numpy as np
import ml_dtypes
from contextlib import ExitStack

import jax
import concourse.bass as bass
import concourse.bass2jax as b2j
import concourse.mybir as mybir
import concourse.tile as tile

N_CORES = 8
N, C, H, W = 16, 3, 256, 256
ROWS = N * C                    # 48
PIX = H * W                     # 65536
P_SLICE = PIX // N_CORES        # 8192 pixels per core
PTILES = P_SLICE // 128         # 64
BINS = 2048
N_RADII, N_ANGLES = 32, 64
OUT_H, OUT_W = 128, 128
XCOLS = 112                     # [hi(48) | pad(16) | lo(48)]: lo at partition 64
LO_OFF = 64                     # PSUM reads must start 32-aligned
NCHUNK = BINS // 512            # 4 PSUM accumulators of width 512

BF16 = ml_dtypes.bfloat16

_PROG = {}
LAST = {"exec_time_ns": None, "results": None}


class SplitWaitTileContext(tile.TileContext):
    """The walrus build in this container rejects instructions carrying more
    than one sync-wait ("Too many sync wait commands", CoreV3GenImpl).  Tile's
    tail drain stacks one wait per live proc, so emit those waits as
    standalone single-wait instructions ahead of a bare drain instead."""

    def _drain_and_barrier(self, tick_clock, wait_clock):
        from concourse.vector_clock import ScopedClock

        nc = self.nc
        collector = nc.sync.nop()
        wait_clock.add_sem_waits(
            collector.ins, ScopedClock({None: tick_clock.global_clock})
        )
        si = collector.ins.sync_info
        waits = list(si.on_wait) if si is not None else []
        if si is not None:
            si.on_wait = waits[:1]
        for w in waits[1:]:
            n = nc.sync.nop()
            nsi = n.ins.sync_info
            if nsi is None:
                n.ins.sync_info = mybir.SyncInfo(on_update=[], on_wait=[w])
            else:
                nsi.on_wait = [w]
        nc.sync.drain()
        nc.all_engine_barrier()
        assert self.sems is not None
        popped = nc._tile_sem_poison_stack.pop()
        assert popped is self._sem_poison
        nc.clear_and_free_semaphores(list(self.sems.allocated().values()))
        nc.all_engine_barrier()


def _split_multi_waits(nc):
    """walrus here allows at most one sync-wait per instruction: move excess
    waits onto same-engine NoOp instructions inserted just before."""
    n = 0
    for fn in nc.m.functions:
        for blk in fn.blocks:
            new = []
            for inst in blk.instructions:
                si = inst.sync_info
                waits = list(si.on_wait) if si is not None else []
                if len(waits) > 1:
                    for w in waits[:-1]:
                        n += 1
                        nop = mybir.InstNoOp(
                            name=f"splitw-{n}",
                            engine=inst.engine,
                            ins=[],
                            outs=[],
                            sync_info=mybir.SyncInfo(on_wait=[w], on_update=[]),
                        )
                        new.append(nop)
                    si.on_wait = waits[-1:]
                new.append(inst)
            blk.instructions[:] = new
    return n


def _build_program(reps=1, split_waits=True):
    """reps>1 repeats the whole pipeline in one NEFF (used only for timing).
    split_waits=False skips the multi-wait legalization (CoreSim chokes on
    the inserted NoOps; hardware/walrus needs them)."""
    nc = bass.Bass(
        "TRN2",
        target_bir_lowering=False,
        debug=False,
        num_devices=N_CORES,
        enable_partition_id=False,
    )
    masks_in = nc.dram_tensor(
        "masks_t", [P_SLICE, BINS], mybir.dt.bfloat16, kind="ExternalInput"
    ).ap()
    x_in = nc.dram_tensor(
        "x_hl", [128, PTILES * XCOLS], mybir.dt.bfloat16, kind="ExternalInput"
    ).ap()
    ws_out = nc.dram_tensor(
        "ws_part", [ROWS, BINS], mybir.dt.float32, kind="ExternalOutput"
    ).ap()

    with ExitStack() as ctx:
        tc = ctx.enter_context(SplitWaitTileContext(nc))
        xp = ctx.enter_context(tc.tile_pool(name="xp", bufs=2))
        mp = ctx.enter_context(tc.tile_pool(name="mp", bufs=6))
        pp = ctx.enter_context(tc.tile_pool(name="pp", bufs=1, space="PSUM"))
        op = ctx.enter_context(tc.tile_pool(name="op", bufs=2))

        for r in range(reps):
            # x (hi|lo split, partition-major packed) stays SBUF-resident.
            x_sb = xp.tile([128, PTILES * XCOLS], mybir.dt.bfloat16, tag="x")
            nc.gpsimd.dma_start(x_sb[:], x_in[:])

            psums = [
                pp.tile(
                    [XCOLS, 512], mybir.dt.float32, tag=f"ps{j}", name=f"ps{r}_{j}"
                )
                for j in range(NCHUNK)
            ]

            for t in range(PTILES):
                mt = mp.tile([128, BINS], mybir.dt.bfloat16, tag="m", name=f"mt{r}_{t}")
                eng = nc.sync if t % 2 == 0 else nc.scalar
                eng.dma_start(mt[:], masks_in[t * 128 : (t + 1) * 128, :])
                for j in range(NCHUNK):
                    nc.tensor.matmul(
                        psums[j][:, :],
                        lhsT=x_sb[:, t * XCOLS : (t + 1) * XCOLS],
                        rhs=mt[:, j * 512 : (j + 1) * 512],
                        start=(t == 0),
                        stop=(t == PTILES - 1),
                    )

            out_sb = op.tile([ROWS, BINS], mybir.dt.float32, tag="o", name=f"out{r}")
            for j in range(NCHUNK):
                # only one non-scalar input may come from PSUM per instruction
                nc.scalar.copy(
                    out_sb[:, j * 512 : (j + 1) * 512], psums[j][0:ROWS, :]
                )
                nc.vector.tensor_add(
                    out_sb[:, j * 512 : (j + 1) * 512],
                    out_sb[:, j * 512 : (j + 1) * 512],
                    psums[j][LO_OFF : LO_OFF + ROWS, :],
                )
            nc.sync.dma_start(ws_out[:], out_sb[:])
    if split_waits:
        _split_multi_waits(nc)
    return nc


def get_program(reps=1):
    if reps not in _PROG:
        _PROG[reps] = _build_program(reps)
    return _PROG[reps]


def _split_hilo(a_f32):
    """Exact split a = hi + lo with hi, lo representable in bf16."""
    hi = a_f32.astype(BF16)
    lo = (a_f32 - hi.astype(np.float32)).astype(BF16)
    return hi, lo


def make_in_maps(x, masks):
    """Host-side sharding/pre-packing of the full inputs into 8 per-core maps."""
    xs = np.ascontiguousarray(x.reshape(ROWS, PIX).T)      # [65536, 48] f32
    hi, lo = _split_hilo(xs)
    xhl = np.zeros((PIX, XCOLS), dtype=BF16)               # [65536, 112] bf16
    xhl[:, :ROWS] = hi
    xhl[:, LO_OFF : LO_OFF + ROWS] = lo

    masks_bf = masks.astype(BF16)                          # [2048, 65536]

    in_maps = []
    for c in range(N_CORES):
        sl = slice(c * P_SLICE, (c + 1) * P_SLICE)
        m_slab = np.ascontiguousarray(masks_bf[:, sl].T)   # [8192, 2048] bf16
        x_slab = np.ascontiguousarray(
            xhl[sl]
            .reshape(PTILES, 128, XCOLS)
            .transpose(1, 0, 2)
            .reshape(128, PTILES * XCOLS)
        )
        in_maps.append({"masks_t": m_slab, "x_hl": x_slab})
    return in_maps


def _program_io(nc):
    partition_name = nc.partition_id_tensor.name if nc.partition_id_tensor else None
    in_names, out_names, out_avals = [], [], []
    for alloc in nc.m.functions[0].allocations:
        if not isinstance(alloc, mybir.MemoryLocationSet):
            continue
        name = alloc.memorylocations[0].name
        if alloc.kind == "ExternalInput":
            if name != partition_name:
                in_names.append(name)
        elif alloc.kind == "ExternalOutput":
            out_names.append(name)
            out_avals.append(
                jax.core.ShapedArray(
                    tuple(alloc.tensor_shape), mybir.dt.np(alloc.dtype)
                )
            )
    return partition_name, in_names, out_names, out_avals


def make_runner(nc):
    """Build a per-core async runner for the program.

    run_bass_kernel_spmd's axon multi-core path (shard_map) lowers to
    multi-computation HLO that neuronx_cc_hook rejects in this jax version,
    so we dispatch one single-device jit per core (the working path) and rely
    on jax async dispatch for the cores to run concurrently.
    """
    b2j.install_neuronx_cc_hook()
    partition_name, in_names, out_names, out_avals = _program_io(nc)
    all_in = list(in_names) + list(out_names)
    if partition_name is not None:
        all_in.append(partition_name)
    zero_outs = [np.zeros(a.shape, a.dtype) for a in out_avals]

    def _body(*args):
        operands = list(args)
        if partition_name is not None:
            operands.append(b2j.partition_id_tensor())
        return tuple(
            b2j._bass_exec_p.bind(
                *operands,
                out_avals=tuple(out_avals),
                in_names=tuple(all_in),
                out_names=tuple(out_names),
                lowering_input_output_aliases=(),
                sim_require_finite=True,
                sim_require_nnan=True,
                nc=nc,
            )
        )

    fn = jax.jit(_body, keep_unused=True)

    def put_core_args(core, in_map):
        dev = jax.devices()[core]
        args = [jax.device_put(np.asarray(in_map[n]), dev) for n in in_names]
        args += [jax.device_put(z, dev) for z in zero_outs]
        return args

    def dispatch(core_args):
        """Enqueue all cores asynchronously, then block; returns per-core dicts."""
        outs = [fn(*args) for args in core_args]
        jax.block_until_ready(outs)
        return [
            {name: np.asarray(o[i]) for i, name in enumerate(out_names)}
            for o in outs
        ]

    return fn, put_core_args, dispatch


def get_runner(reps=1):
    key = ("runner", reps)
    if key not in _PROG:
        _PROG[key] = make_runner(get_program(reps))
    return _PROG[key]


def device_ws(x, masks):
    """Run the sharded device kernel; return full ws = xs @ masks.T as f64."""
    _, put_core_args, dispatch = get_runner()
    in_maps = make_in_maps(x, masks)
    core_args = [put_core_args(c, m) for c, m in enumerate(in_maps)]
    results = dispatch(core_args)
    LAST["results"] = results
    parts = np.stack([r["ws_part"] for r in results])  # [8, 48, 2048]
    return parts.astype(np.float64).sum(axis=0)


def _resize_matrix(in_size, out_size):
    """Row-interp matrix replicating jax.image.resize 'bilinear' upsampling."""
    scale = out_size / in_size
    sample = (np.arange(out_size, dtype=np.float64) + 0.5) / scale - 0.5
    w = 1.0 - np.abs(sample[:, None] - np.arange(in_size, dtype=np.float64)[None, :])
    w = np.clip(w, 0.0, None)
    w = w / w.sum(axis=1, keepdims=True)
    return w  # [out, in] f64


def kernel(x, pooling_masks, pooling_mask_counts):
    x = np.asarray(x, dtype=np.float32)
    masks = np.asarray(pooling_masks, dtype=np.float32)
    counts = np.asarray(pooling_mask_counts, dtype=np.float32)

    exact_binary = bool(np.all((masks == 0.0) | (masks == 1.0)))
    if exact_binary:
        ws = device_ws(x, masks)
    else:
        # Fallback for non-{0,1} masks: bf16 cast would be lossy, so split the
        # masks exactly into hi+lo bf16 and run the same kernel twice.
        mhi = masks.astype(BF16).astype(np.float32)
        mlo = masks - mhi
        ws = device_ws(x, mhi) + device_ws(x, mlo)

    out = ws.reshape(N, C, N_RADII, N_ANGLES) / counts.reshape(
        1, 1, N_RADII, N_ANGLES
    ).astype(np.float64)
    out = np.pad(out, ((0, 0), (0, 0), (0, 0), (1, 1)), mode="reflect")
    a_h = _resize_matrix(N_RADII, OUT_H)          # [128, 32]
    a_w = _resize_matrix(N_ANGLES + 2, OUT_W)     # [128, 66]
    out = np.matmul(a_h, out)                     # [n, c, 128, 66]
    out = np.matmul(out, a_w.T)                   # [n, c, 128, 128]
    return out.astype(np.float32)


# revision 14
# speedup vs baseline: 1.0951x; 1.0951x over previous
"""Trainium2 Bass kernel for nn_PolarTransform (histogram binning + bilinear resize).

Computation (matches the reference):
    xs  = x.reshape(48, 65536)
    ws  = xs @ pooling_masks.T                 # [48, 2048]  <- the heavy part
    out = ws.reshape(16,3,32,64) / counts
    out = reflect-pad angles (1,1) -> [16,3,32,66]
    out = bilinear resize -> [16,3,128,128]

Strategy:
  * The dominant cost is streaming pooling_masks ([2048, 65536] f32 = 512 MB).
    Mask values are exactly {0.0, 1.0}, so a bf16 cast is lossless and halves
    the HBM traffic.  x is split exactly into hi+lo bf16 (x == hi + lo in f32),
    keeping ~f32 precision while using the fast bf16 matmul path (1 cyc/row
    vs 4 cyc/row for f32 on the PE).
  * Shard the 65536-pixel contraction dim across the 8 cores (8192 px each).
    Each core streams its [8192, 2048] bf16 transposed-mask slab (32 MB) and
    accumulates a partial ws [48, 2048] in PSUM.  Host sums the 8 partials
    and applies the tiny counts/pad/resize epilogue ([48, 2048] -> output).
  * Mask slabs are pre-transposed on the host (constant-buffer pre-packing) so
    the device streams them at DMA line rate with the contraction dim on
    SBUF partitions, as the PE requires.
"""

import numpy as np
import ml_dtypes
from contextlib import ExitStack

import jax
import concourse.bass as bass
import concourse.bass2jax as b2j
import concourse.mybir as mybir
import concourse.tile as tile

N_CORES = 8
N, C, H, W = 16, 3, 256, 256
ROWS = N * C                    # 48
PIX = H * W                     # 65536
P_SLICE = PIX // N_CORES        # 8192 pixels per core
PTILES = P_SLICE // 128         # 64
BINS = 2048
N_RADII, N_ANGLES = 32, 64
OUT_H, OUT_W = 128, 128
XCOLS = 112                     # [hi(48) | pad(16) | lo(48)]: lo at partition 64
LO_OFF = 64                     # PSUM reads must start 32-aligned
NCHUNK = BINS // 512            # 4 PSUM accumulators of width 512

BF16 = ml_dtypes.bfloat16

_PROG = {}
LAST = {"exec_time_ns": None, "results": None}


class SplitWaitTileContext(tile.TileContext):
    """The walrus build in this container rejects instructions carrying more
    than one sync-wait ("Too many sync wait commands", CoreV3GenImpl).  Tile's
    tail drain stacks one wait per live proc, so emit those waits as
    standalone single-wait instructions ahead of a bare drain instead."""

    def _drain_and_barrier(self, tick_clock, wait_clock):
        from concourse.vector_clock import ScopedClock

        nc = self.nc
        collector = nc.sync.nop()
        wait_clock.add_sem_waits(
            collector.ins, ScopedClock({None: tick_clock.global_clock})
        )
        si = collector.ins.sync_info
        waits = list(si.on_wait) if si is not None else []
        if si is not None:
            si.on_wait = waits[:1]
        for w in waits[1:]:
            n = nc.sync.nop()
            nsi = n.ins.sync_info
            if nsi is None:
                n.ins.sync_info = mybir.SyncInfo(on_update=[], on_wait=[w])
            else:
                nsi.on_wait = [w]
        nc.sync.drain()
        nc.all_engine_barrier()
        assert self.sems is not None
        popped = nc._tile_sem_poison_stack.pop()
        assert popped is self._sem_poison
        nc.clear_and_free_semaphores(list(self.sems.allocated().values()))
        nc.all_engine_barrier()


def _split_multi_waits(nc):
    """walrus here allows at most one sync-wait per instruction: move excess
    waits onto same-engine NoOp instructions inserted just before."""
    n = 0
    for fn in nc.m.functions:
        for blk in fn.blocks:
            new = []
            for inst in blk.instructions:
                si = inst.sync_info
                waits = list(si.on_wait) if si is not None else []
                if len(waits) > 1:
                    for w in waits[:-1]:
                        n += 1
                        nop = mybir.InstNoOp(
                            name=f"splitw-{n}",
                            engine=inst.engine,
                            ins=[],
                            outs=[],
                            sync_info=mybir.SyncInfo(on_wait=[w], on_update=[]),
                        )
                        new.append(nop)
                    si.on_wait = waits[-1:]
                new.append(inst)
            blk.instructions[:] = new
    return n


def _build_program(reps=1, split_waits=True):
    """reps>1 repeats the whole pipeline in one NEFF (used only for timing).
    split_waits=False skips the multi-wait legalization (CoreSim chokes on
    the inserted NoOps; hardware/walrus needs them)."""
    nc = bass.Bass(
        "TRN2",
        target_bir_lowering=False,
        debug=False,
        num_devices=N_CORES,
        enable_partition_id=False,
    )
    masks_in = nc.dram_tensor(
        "masks_t", [P_SLICE, BINS], mybir.dt.bfloat16, kind="ExternalInput"
    ).ap()
    x_in = nc.dram_tensor(
        "x_hl", [128, PTILES * XCOLS], mybir.dt.bfloat16, kind="ExternalInput"
    ).ap()
    ws_out = nc.dram_tensor(
        "ws_part", [ROWS, BINS], mybir.dt.float32, kind="ExternalOutput"
    ).ap()

    with ExitStack() as ctx:
        tc = ctx.enter_context(SplitWaitTileContext(nc))
        xp = ctx.enter_context(tc.tile_pool(name="xp", bufs=2))
        mp = ctx.enter_context(tc.tile_pool(name="mp", bufs=12))
        pp = ctx.enter_context(tc.tile_pool(name="pp", bufs=1, space="PSUM"))
        op = ctx.enter_context(tc.tile_pool(name="op", bufs=2))

        for r in range(reps):
            # x (hi|lo split, partition-major packed) stays SBUF-resident.
            x_sb = xp.tile([128, PTILES * XCOLS], mybir.dt.bfloat16, tag="x")
            nc.gpsimd.dma_start(x_sb[:], x_in[:])

            psums = [
                pp.tile(
                    [XCOLS, 512], mybir.dt.float32, tag=f"ps{j}", name=f"ps{r}_{j}"
                )
                for j in range(NCHUNK)
            ]

            for t in range(PTILES):
                mt = mp.tile([128, BINS], mybir.dt.bfloat16, tag="m", name=f"mt{r}_{t}")
                eng = nc.sync if t % 2 == 0 else nc.scalar
                eng.dma_start(mt[:], masks_in[t * 128 : (t + 1) * 128, :])
                for j in range(NCHUNK):
                    nc.tensor.matmul(
                        psums[j][:, :],
                        lhsT=x_sb[:, t * XCOLS : (t + 1) * XCOLS],
                        rhs=mt[:, j * 512 : (j + 1) * 512],
                        start=(t == 0),
                        stop=(t == PTILES - 1),
                    )

            out_sb = op.tile([ROWS, BINS], mybir.dt.float32, tag="o", name=f"out{r}")
            for j in range(NCHUNK):
                # only one non-scalar input may come from PSUM per instruction
                nc.scalar.copy(
                    out_sb[:, j * 512 : (j + 1) * 512], psums[j][0:ROWS, :]
                )
                nc.vector.tensor_add(
                    out_sb[:, j * 512 : (j + 1) * 512],
                    out_sb[:, j * 512 : (j + 1) * 512],
                    psums[j][LO_OFF : LO_OFF + ROWS, :],
                )
            nc.sync.dma_start(ws_out[:], out_sb[:])
    if split_waits:
        _split_multi_waits(nc)
    return nc


def get_program(reps=1):
    if reps not in _PROG:
        _PROG[reps] = _build_program(reps)
    return _PROG[reps]


def _split_hilo(a_f32):
    """Exact split a = hi + lo with hi, lo representable in bf16."""
    hi = a_f32.astype(BF16)
    lo = (a_f32 - hi.astype(np.float32)).astype(BF16)
    return hi, lo


def make_in_maps(x, masks):
    """Host-side sharding/pre-packing of the full inputs into 8 per-core maps."""
    xs = np.ascontiguousarray(x.reshape(ROWS, PIX).T)      # [65536, 48] f32
    hi, lo = _split_hilo(xs)
    xhl = np.zeros((PIX, XCOLS), dtype=BF16)               # [65536, 112] bf16
    xhl[:, :ROWS] = hi
    xhl[:, LO_OFF : LO_OFF + ROWS] = lo

    masks_bf = masks.astype(BF16)                          # [2048, 65536]

    in_maps = []
    for c in range(N_CORES):
        sl = slice(c * P_SLICE, (c + 1) * P_SLICE)
        m_slab = np.ascontiguousarray(masks_bf[:, sl].T)   # [8192, 2048] bf16
        x_slab = np.ascontiguousarray(
            xhl[sl]
            .reshape(PTILES, 128, XCOLS)
            .transpose(1, 0, 2)
            .reshape(128, PTILES * XCOLS)
        )
        in_maps.append({"masks_t": m_slab, "x_hl": x_slab})
    return in_maps


def _program_io(nc):
    partition_name = nc.partition_id_tensor.name if nc.partition_id_tensor else None
    in_names, out_names, out_avals = [], [], []
    for alloc in nc.m.functions[0].allocations:
        if not isinstance(alloc, mybir.MemoryLocationSet):
            continue
        name = alloc.memorylocations[0].name
        if alloc.kind == "ExternalInput":
            if name != partition_name:
                in_names.append(name)
        elif alloc.kind == "ExternalOutput":
            out_names.append(name)
            out_avals.append(
                jax.core.ShapedArray(
                    tuple(alloc.tensor_shape), mybir.dt.np(alloc.dtype)
                )
            )
    return partition_name, in_names, out_names, out_avals


def make_runner(nc):
    """Build a per-core async runner for the program.

    run_bass_kernel_spmd's axon multi-core path (shard_map) lowers to
    multi-computation HLO that neuronx_cc_hook rejects in this jax version,
    so we dispatch one single-device jit per core (the working path) and rely
    on jax async dispatch for the cores to run concurrently.
    """
    b2j.install_neuronx_cc_hook()
    partition_name, in_names, out_names, out_avals = _program_io(nc)
    all_in = list(in_names) + list(out_names)
    if partition_name is not None:
        all_in.append(partition_name)
    zero_outs = [np.zeros(a.shape, a.dtype) for a in out_avals]

    def _body(*args):
        operands = list(args)
        if partition_name is not None:
            operands.append(b2j.partition_id_tensor())
        return tuple(
            b2j._bass_exec_p.bind(
                *operands,
                out_avals=tuple(out_avals),
                in_names=tuple(all_in),
                out_names=tuple(out_names),
                lowering_input_output_aliases=(),
                sim_require_finite=True,
                sim_require_nnan=True,
                nc=nc,
            )
        )

    fn = jax.jit(_body, keep_unused=True)

    def put_core_args(core, in_map):
        dev = jax.devices()[core]
        args = [jax.device_put(np.asarray(in_map[n]), dev) for n in in_names]
        args += [jax.device_put(z, dev) for z in zero_outs]
        return args

    def dispatch(core_args):
        """Enqueue all cores asynchronously, then block; returns per-core dicts."""
        outs = [fn(*args) for args in core_args]
        jax.block_until_ready(outs)
        return [
            {name: np.asarray(o[i]) for i, name in enumerate(out_names)}
            for o in outs
        ]

    return fn, put_core_args, dispatch


def get_runner(reps=1):
    key = ("runner", reps)
    if key not in _PROG:
        _PROG[key] = make_runner(get_program(reps))
    return _PROG[key]


def device_ws(x, masks):
    """Run the sharded device kernel; return full ws = xs @ masks.T as f64."""
    _, put_core_args, dispatch = get_runner()
    in_maps = make_in_maps(x, masks)
    core_args = [put_core_args(c, m) for c, m in enumerate(in_maps)]
    results = dispatch(core_args)
    LAST["results"] = results
    parts = np.stack([r["ws_part"] for r in results])  # [8, 48, 2048]
    return parts.astype(np.float64).sum(axis=0)


def _resize_matrix(in_size, out_size):
    """Row-interp matrix replicating jax.image.resize 'bilinear' upsampling."""
    scale = out_size / in_size
    sample = (np.arange(out_size, dtype=np.float64) + 0.5) / scale - 0.5
    w = 1.0 - np.abs(sample[:, None] - np.arange(in_size, dtype=np.float64)[None, :])
    w = np.clip(w, 0.0, None)
    w = w / w.sum(axis=1, keepdims=True)
    return w  # [out, in] f64


def kernel(x, pooling_masks, pooling_mask_counts):
    x = np.asarray(x, dtype=np.float32)
    masks = np.asarray(pooling_masks, dtype=np.float32)
    counts = np.asarray(pooling_mask_counts, dtype=np.float32)

    exact_binary = bool(np.all((masks == 0.0) | (masks == 1.0)))
    if exact_binary:
        ws = device_ws(x, masks)
    else:
        # Fallback for non-{0,1} masks: bf16 cast would be lossy, so split the
        # masks exactly into hi+lo bf16 and run the same kernel twice.
        mhi = masks.astype(BF16).astype(np.float32)
        mlo = masks - mhi
        ws = device_ws(x, mhi) + device_ws(x, mlo)

    out = ws.reshape(N, C, N_RADII, N_ANGLES) / counts.reshape(
        1, 1, N_RADII, N_ANGLES
    ).astype(np.float64)
    out = np.pad(out, ((0, 0), (0, 0), (0, 0), (1, 1)), mode="reflect")
    a_h = _resize_matrix(N_RADII, OUT_H)          # [128, 32]
    a_w = _resize_matrix(N_ANGLES + 2, OUT_W)     # [128, 66]
    out = np.matmul(a_h, out)                     # [n, c, 128, 66]
    out = np.matmul(out, a_w.T)                   # [n, c, 128, 128]
    return out.astype(np.float32)


# revision 15
# speedup vs baseline: 1.1745x; 1.0725x over previous
"""Trainium2 Bass kernel for nn_PolarTransform (histogram binning + bilinear resize).

Computation (matches the reference):
    xs  = x.reshape(48, 65536)
    ws  = xs @ pooling_masks.T                 # [48, 2048]  <- the heavy part
    out = ws.reshape(16,3,32,64) / counts
    out = reflect-pad angles (1,1) -> [16,3,32,66]
    out = bilinear resize -> [16,3,128,128]

Strategy:
  * The dominant cost is streaming pooling_masks ([2048, 65536] f32 = 512 MB).
    Mask values are exactly {0.0, 1.0}, so an fp8e4m3 cast is lossless and
    quarters the HBM traffic (moving operand stays full-rate on the PE).  x is split exactly into hi+lo bf16 (x == hi + lo in f32),
    keeping ~f32 precision while using the fast bf16 matmul path (1 cyc/row
    vs 4 cyc/row for f32 on the PE).
  * Shard the 65536-pixel contraction dim across the 8 cores (8192 px each).
    Each core streams its [8192, 2048] bf16 transposed-mask slab (32 MB) and
    accumulates a partial ws [48, 2048] in PSUM.  Host sums the 8 partials
    and applies the tiny counts/pad/resize epilogue ([48, 2048] -> output).
  * Mask slabs are pre-transposed on the host (constant-buffer pre-packing) so
    the device streams them at DMA line rate with the contraction dim on
    SBUF partitions, as the PE requires.
"""

import numpy as np
import ml_dtypes
from contextlib import ExitStack

import jax
import concourse.bass as bass
import concourse.bass2jax as b2j
import concourse.mybir as mybir
import concourse.tile as tile

N_CORES = 8
N, C, H, W = 16, 3, 256, 256
ROWS = N * C                    # 48
PIX = H * W                     # 65536
P_SLICE = PIX // N_CORES        # 8192 pixels per core
PTILES = P_SLICE // 128         # 64
BINS = 2048
N_RADII, N_ANGLES = 32, 64
OUT_H, OUT_W = 128, 128
XCOLS = 112                     # [hi(48) | pad(16) | lo(48)]: lo at partition 64
LO_OFF = 64                     # PSUM reads must start 32-aligned
NCHUNK = BINS // 512            # 4 PSUM accumulators of width 512

BF16 = ml_dtypes.bfloat16
FP8 = ml_dtypes.float8_e4m3     # 0.0/1.0 exactly representable

_PROG = {}
LAST = {"exec_time_ns": None, "results": None}


class SplitWaitTileContext(tile.TileContext):
    """The walrus build in this container rejects instructions carrying more
    than one sync-wait ("Too many sync wait commands", CoreV3GenImpl).  Tile's
    tail drain stacks one wait per live proc, so emit those waits as
    standalone single-wait instructions ahead of a bare drain instead."""

    def _drain_and_barrier(self, tick_clock, wait_clock):
        from concourse.vector_clock import ScopedClock

        nc = self.nc
        collector = nc.sync.nop()
        wait_clock.add_sem_waits(
            collector.ins, ScopedClock({None: tick_clock.global_clock})
        )
        si = collector.ins.sync_info
        waits = list(si.on_wait) if si is not None else []
        if si is not None:
            si.on_wait = waits[:1]
        for w in waits[1:]:
            n = nc.sync.nop()
            nsi = n.ins.sync_info
            if nsi is None:
                n.ins.sync_info = mybir.SyncInfo(on_update=[], on_wait=[w])
            else:
                nsi.on_wait = [w]
        nc.sync.drain()
        nc.all_engine_barrier()
        assert self.sems is not None
        popped = nc._tile_sem_poison_stack.pop()
        assert popped is self._sem_poison
        nc.clear_and_free_semaphores(list(self.sems.allocated().values()))
        nc.all_engine_barrier()


def _split_multi_waits(nc):
    """walrus here allows at most one sync-wait per instruction: move excess
    waits onto same-engine NoOp instructions inserted just before."""
    n = 0
    for fn in nc.m.functions:
        for blk in fn.blocks:
            new = []
            for inst in blk.instructions:
                si = inst.sync_info
                waits = list(si.on_wait) if si is not None else []
                if len(waits) > 1:
                    for w in waits[:-1]:
                        n += 1
                        nop = mybir.InstNoOp(
                            name=f"splitw-{n}",
                            engine=inst.engine,
                            ins=[],
                            outs=[],
                            sync_info=mybir.SyncInfo(on_wait=[w], on_update=[]),
                        )
                        new.append(nop)
                    si.on_wait = waits[-1:]
                new.append(inst)
            blk.instructions[:] = new
    return n


def _build_program(reps=1, split_waits=True):
    """reps>1 repeats the whole pipeline in one NEFF (used only for timing).
    split_waits=False skips the multi-wait legalization (CoreSim chokes on
    the inserted NoOps; hardware/walrus needs them)."""
    nc = bass.Bass(
        "TRN2",
        target_bir_lowering=False,
        debug=False,
        num_devices=N_CORES,
        enable_partition_id=False,
    )
    masks_in = nc.dram_tensor(
        "masks_t", [P_SLICE, BINS], mybir.dt.float8e4, kind="ExternalInput"
    ).ap()
    x_in = nc.dram_tensor(
        "x_hl", [128, PTILES * XCOLS], mybir.dt.bfloat16, kind="ExternalInput"
    ).ap()
    ws_out = nc.dram_tensor(
        "ws_part", [ROWS, BINS], mybir.dt.float32, kind="ExternalOutput"
    ).ap()

    with ExitStack() as ctx:
        tc = ctx.enter_context(SplitWaitTileContext(nc))
        xp = ctx.enter_context(tc.tile_pool(name="xp", bufs=2))
        mp = ctx.enter_context(tc.tile_pool(name="mp", bufs=12))
        pp = ctx.enter_context(tc.tile_pool(name="pp", bufs=1, space="PSUM"))
        op = ctx.enter_context(tc.tile_pool(name="op", bufs=2))

        for r in range(reps):
            # x (hi|lo split, partition-major packed) stays SBUF-resident.
            x_sb = xp.tile([128, PTILES * XCOLS], mybir.dt.bfloat16, tag="x")
            nc.gpsimd.dma_start(x_sb[:], x_in[:])

            psums = [
                pp.tile(
                    [XCOLS, 512], mybir.dt.float32, tag=f"ps{j}", name=f"ps{r}_{j}"
                )
                for j in range(NCHUNK)
            ]

            for t in range(PTILES):
                mt = mp.tile([128, BINS], mybir.dt.float8e4, tag="m", name=f"mt{r}_{t}")
                eng = nc.sync if t % 2 == 0 else nc.scalar
                eng.dma_start(mt[:], masks_in[t * 128 : (t + 1) * 128, :])
                for j in range(NCHUNK):
                    nc.tensor.matmul(
                        psums[j][:, :],
                        lhsT=x_sb[:, t * XCOLS : (t + 1) * XCOLS],
                        rhs=mt[:, j * 512 : (j + 1) * 512],
                        start=(t == 0),
                        stop=(t == PTILES - 1),
                    )

            out_sb = op.tile([ROWS, BINS], mybir.dt.float32, tag="o", name=f"out{r}")
            for j in range(NCHUNK):
                # only one non-scalar input may come from PSUM per instruction
                nc.scalar.copy(
                    out_sb[:, j * 512 : (j + 1) * 512], psums[j][0:ROWS, :]
                )
                nc.vector.tensor_add(
                    out_sb[:, j * 512 : (j + 1) * 512],
                    out_sb[:, j * 512 : (j + 1) * 512],
                    psums[j][LO_OFF : LO_OFF + ROWS, :],
                )
            nc.sync.dma_start(ws_out[:], out_sb[:])
    if split_waits:
        _split_multi_waits(nc)
    return nc


def get_program(reps=1):
    if reps not in _PROG:
        _PROG[reps] = _build_program(reps)
    return _PROG[reps]


def _split_hilo(a_f32):
    """Exact split a = hi + lo with hi, lo representable in bf16."""
    hi = a_f32.astype(BF16)
    lo = (a_f32 - hi.astype(np.float32)).astype(BF16)
    return hi, lo


def make_in_maps(x, masks):
    """Host-side sharding/pre-packing of the full inputs into 8 per-core maps."""
    xs = np.ascontiguousarray(x.reshape(ROWS, PIX).T)      # [65536, 48] f32
    hi, lo = _split_hilo(xs)
    xhl = np.zeros((PIX, XCOLS), dtype=BF16)               # [65536, 112] bf16
    xhl[:, :ROWS] = hi
    xhl[:, LO_OFF : LO_OFF + ROWS] = lo

    masks_bf = masks.astype(FP8)                           # [2048, 65536] e4m3

    in_maps = []
    for c in range(N_CORES):
        sl = slice(c * P_SLICE, (c + 1) * P_SLICE)
        m_slab = np.ascontiguousarray(masks_bf[:, sl].T)   # [8192, 2048] bf16
        x_slab = np.ascontiguousarray(
            xhl[sl]
            .reshape(PTILES, 128, XCOLS)
            .transpose(1, 0, 2)
            .reshape(128, PTILES * XCOLS)
        )
        in_maps.append({"masks_t": m_slab, "x_hl": x_slab})
    return in_maps


def _program_io(nc):
    partition_name = nc.partition_id_tensor.name if nc.partition_id_tensor else None
    in_names, out_names, out_avals = [], [], []
    for alloc in nc.m.functions[0].allocations:
        if not isinstance(alloc, mybir.MemoryLocationSet):
            continue
        name = alloc.memorylocations[0].name
        if alloc.kind == "ExternalInput":
            if name != partition_name:
                in_names.append(name)
        elif alloc.kind == "ExternalOutput":
            out_names.append(name)
            out_avals.append(
                jax.core.ShapedArray(
                    tuple(alloc.tensor_shape), mybir.dt.np(alloc.dtype)
                )
            )
    return partition_name, in_names, out_names, out_avals


def make_runner(nc):
    """Build a per-core async runner for the program.

    run_bass_kernel_spmd's axon multi-core path (shard_map) lowers to
    multi-computation HLO that neuronx_cc_hook rejects in this jax version,
    so we dispatch one single-device jit per core (the working path) and rely
    on jax async dispatch for the cores to run concurrently.
    """
    b2j.install_neuronx_cc_hook()
    partition_name, in_names, out_names, out_avals = _program_io(nc)
    all_in = list(in_names) + list(out_names)
    if partition_name is not None:
        all_in.append(partition_name)
    zero_outs = [np.zeros(a.shape, a.dtype) for a in out_avals]

    def _body(*args):
        operands = list(args)
        if partition_name is not None:
            operands.append(b2j.partition_id_tensor())
        return tuple(
            b2j._bass_exec_p.bind(
                *operands,
                out_avals=tuple(out_avals),
                in_names=tuple(all_in),
                out_names=tuple(out_names),
                lowering_input_output_aliases=(),
                sim_require_finite=True,
                sim_require_nnan=True,
                nc=nc,
            )
        )

    fn = jax.jit(_body, keep_unused=True)

    def put_core_args(core, in_map):
        dev = jax.devices()[core]
        args = [jax.device_put(np.asarray(in_map[n]), dev) for n in in_names]
        args += [jax.device_put(z, dev) for z in zero_outs]
        return args

    def dispatch(core_args):
        """Enqueue all cores asynchronously, then block; returns per-core dicts."""
        outs = [fn(*args) for args in core_args]
        jax.block_until_ready(outs)
        return [
            {name: np.asarray(o[i]) for i, name in enumerate(out_names)}
            for o in outs
        ]

    return fn, put_core_args, dispatch


def get_runner(reps=1):
    key = ("runner", reps)
    if key not in _PROG:
        _PROG[key] = make_runner(get_program(reps))
    return _PROG[key]


def device_ws(x, masks):
    """Run the sharded device kernel; return full ws = xs @ masks.T as f64."""
    _, put_core_args, dispatch = get_runner()
    in_maps = make_in_maps(x, masks)
    core_args = [put_core_args(c, m) for c, m in enumerate(in_maps)]
    results = dispatch(core_args)
    LAST["results"] = results
    parts = np.stack([r["ws_part"] for r in results])  # [8, 48, 2048]
    return parts.astype(np.float64).sum(axis=0)


def _resize_matrix(in_size, out_size):
    """Row-interp matrix replicating jax.image.resize 'bilinear' upsampling."""
    scale = out_size / in_size
    sample = (np.arange(out_size, dtype=np.float64) + 0.5) / scale - 0.5
    w = 1.0 - np.abs(sample[:, None] - np.arange(in_size, dtype=np.float64)[None, :])
    w = np.clip(w, 0.0, None)
    w = w / w.sum(axis=1, keepdims=True)
    return w  # [out, in] f64


def kernel(x, pooling_masks, pooling_mask_counts):
    x = np.asarray(x, dtype=np.float32)
    masks = np.asarray(pooling_masks, dtype=np.float32)
    counts = np.asarray(pooling_mask_counts, dtype=np.float32)

    exact_binary = bool(np.all((masks == 0.0) | (masks == 1.0)))
    if exact_binary:
        ws = device_ws(x, masks)
    else:
        # Fallback for non-{0,1} masks: bf16 cast would be lossy, so split the
        # masks exactly into hi+lo bf16 and run the same kernel twice.
        mhi = masks.astype(BF16).astype(np.float32)
        mlo = masks - mhi
        ws = device_ws(x, mhi) + device_ws(x, mlo)

    out = ws.reshape(N, C, N_RADII, N_ANGLES) / counts.reshape(
        1, 1, N_RADII, N_ANGLES
    ).astype(np.float64)
    out = np.pad(out, ((0, 0), (0, 0), (0, 0), (1, 1)), mode="reflect")
    a_h = _resize_matrix(N_RADII, OUT_H)          # [128, 32]
    a_w = _resize_matrix(N_ANGLES + 2, OUT_W)     # [128, 66]
    out = np.matmul(a_h, out)                     # [n, c, 128, 66]
    out = np.matmul(out, a_w.T)                   # [n, c, 128, 128]
    return out.astype(np.float32)
